# revision 10
# baseline (speedup 1.0000x reference)
"""Causal self-attention (GQA + RoPE) Trainium2 kernel, v2 (bf16).

Full-input contract: kernel(**inputs) takes the unsharded tensors and returns
the full [B, T, C] output. Shards over 8 NeuronCores as (batch b) x (kv-head
group g); each core computes the attention output of its 4 query heads for its
batch plus the partial out-projection against its 512 rows of Wo; the host
upcasts the bf16 partials and sums the 4 group partials per batch.

All matmul operands are bf16 (fp32 PSUM accumulation). The host pre-arranges
every input into its final SBUF layout so each tensor loads with one or two
large contiguous DMAs. Phase A runs j-major accumulation passes (16 chunk
matmuls per output tile) so PSUM tiles free up a full pass before they are
reallocated. Phase B emits scores(kt+1) before ones/ot(kt) so the PE never
head-of-line blocks on the exp, and processes head pairs through shared
[128,2,512] PSUM tiles so one ACT instruction exps both heads.
"""

import sys

for _p in ("/opt/trn_rl_repo", "/root/.axon_site/_ro/trn_rl_repo"):
    if _p not in sys.path:
        sys.path.append(_p)

import numpy as np
import ml_dtypes
from contextlib import ExitStack

import concourse.bass as bass
import concourse.bacc as bacc
import concourse.tile as tile
import concourse.mybir as mybir
from concourse.bass_utils import run_bass_kernel_spmd

F32 = mybir.dt.float32
BF16 = mybir.dt.bfloat16
BFNP = ml_dtypes.bfloat16

B, T, C = 2, 2048, 2048
N_HEADS, N_KV_HEADS, HD = 16, 4, 128
G = N_HEADS // N_KV_HEADS  # 4 heads per kv group
GW = G * HD  # 512
N_CORES = 8
TC = 512  # q-block / t-chunk width
NTC = T // TC  # 4
NCC = C // 128  # 16 contraction chunks
MASK_NEG = -1.0e30

_prog_cache = {}


def _build_program():
    nc = bacc.Bacc(
        "TRN2",
        target_bir_lowering=False,
        debug=False,
        enable_asserts=False,
        num_devices=N_CORES,
    )

    xp = nc.dram_tensor("xp", [128, NTC * NCC * TC], BF16, kind="ExternalInput").ap()
    wq = nc.dram_tensor("wq", [128, NCC * GW], BF16, kind="ExternalInput").ap()
    wk = nc.dram_tensor("wk", [128, NCC * HD], BF16, kind="ExternalInput").ap()
    wv = nc.dram_tensor("wv", [128, NCC * HD], BF16, kind="ExternalInput").ap()
    wo = nc.dram_tensor("wo", [128, G * C], BF16, kind="ExternalInput").ap()
    cs = nc.dram_tensor("cs", [128, 2 * T], BF16, kind="ExternalInput").ap()
    maskf = nc.dram_tensor("maskf", [128, 128], F32, kind="ExternalInput").ap()
    consts = nc.dram_tensor("consts", [128, 256], BF16, kind="ExternalInput").ap()
    y = nc.dram_tensor("y", [T, C], BF16, kind="ExternalOutput").ap()

    with tile.TileContext(nc) as tc, ExitStack() as ctx:
        cpool = ctx.enter_context(tc.tile_pool(name="const", bufs=1))
        big = ctx.enter_context(tc.tile_pool(name="big", bufs=1))

        wq_sb = cpool.tile([128, NCC * GW], BF16)
        wk_sb = cpool.tile([128, NCC * HD], BF16)
        wv_sb = cpool.tile([128, NCC * HD], BF16)
        wo_sb = cpool.tile([128, G * C], BF16)
        cs_sb = cpool.tile([128, 2 * T], BF16)
        mask_sb = cpool.tile([128, 128], F32)
        consts_sb = cpool.tile([128, 256], BF16)
        warm_sb = cpool.tile([128, 512], BF16)

        x_sb = big.tile([128, NTC * NCC * TC], BF16)  # per t-chunk, chunk-major
        qt_sb = big.tile([128, G * T], BF16)  # qb-major: qb*2048 + j*512 + t
        kt_sb = big.tile([128, T], BF16)
        v_sb = big.tile([128, T], BF16)  # kt-major blocks of [t, d]

        ident_sb = consts_sb[:, 0:128]
        ones_sb = consts_sb[:, 128:256]

        # ---------------- prologue: warm-up + input DMAs ----------------
        # Weights go on the gpsimd (SWDGE) queue: the scalar/ACT queue must
        # stay free for rope copies (an ACT-queued DMA serializes with its
        # transfer and delays every PSUM-ring free downstream). x streams on
        # sync with fine-grained leading pieces so the first j-pass never
        # outruns the transfer.
        nc.vector.memset(warm_sb[:], 0.0)
        nc.scalar.dma_start(wq_sb[:, 0:GW], wq[:, 0:GW])
        nc.scalar.dma_start(wk_sb[:], wk[:])
        nc.scalar.dma_start(wv_sb[:], wv[:])
        nc.scalar.dma_start(wq_sb[:, GW:], wq[:, GW:])
        nc.scalar.dma_start(cs_sb[:], cs[:])
        nc.scalar.dma_start(mask_sb[:], maskf[:])
        nc.scalar.dma_start(consts_sb[:], consts[:])
        nc.scalar.dma_start(wo_sb[:], wo[:])
        xsplits = [0, 512, 2048, 4096, 8192, 16384, 24576, 32768]
        for a, b_ in zip(xsplits[:-1], xsplits[1:]):
            nc.sync.dma_start(x_sb[:, a:b_], xp[:, a:b_])

        # ---------------- phase A: projections + rope ----------------
        with ExitStack() as pa:
            qt_ps_pool = pa.enter_context(tc.tile_pool(name="qtps", bufs=4, space="PSUM"))
            kv_ps_pool = pa.enter_context(tc.tile_pool(name="kvps", bufs=2, space="PSUM"))
            tp_ps_pool = pa.enter_context(tc.tile_pool(name="tpps", bufs=1, space="PSUM"))
            rp = pa.enter_context(tc.tile_pool(name="rp", bufs=3))

            # warm-up matmuls: no DMA dependency, flip the HAM clock gate
            # while the first input DMAs land. Reuse the qt psum ring.
            for w in range(10):
                wt = qt_ps_pool.tile([128, TC], F32, tag="qtps", name=f"wm{w}")
                nc.tensor.matmul(wt[:], warm_sb[:, 0:128], warm_sb[:], start=True, stop=True)

            for tci in range(NTC):
                ts = slice(tci * TC, (tci + 1) * TC)
                qt_ps = [
                    qt_ps_pool.tile([128, TC], F32, tag="qtps", name=f"qtps{tci}_{j}")
                    for j in range(G)
                ]
                kt_ps = kv_ps_pool.tile([128, TC], F32, tag="kvps", name=f"ktps{tci}")
                vt_ps = kv_ps_pool.tile([128, TC], F32, tag="kvps", name=f"vtps{tci}")

                def xs(ci):
                    base = tci * NCC * TC
                    return x_sb[:, base + ci * TC : base + (ci + 1) * TC]

                # j-major passes with the rope emitted right after each
                # pass: the ACT copy frees the qt psum tile while the next
                # pass computes, and the DVE/swap work spreads evenly.
                for j in range(G):
                    for ci in range(NCC):
                        nc.tensor.matmul(
                            qt_ps[j][:],
                            wq_sb[:, ci * GW + j * HD : ci * GW + (j + 1) * HD],
                            xs(ci),
                            start=(ci == 0),
                            stop=(ci == NCC - 1),
                        )
                    q_raw = rp.tile([128, TC], BF16, tag="qraw", name=f"qraw{tci}_{j}")
                    nc.scalar.copy(q_raw[:], qt_ps[j][:])
                    t1 = rp.tile([128, TC], BF16, tag="t1", name=f"t1_{tci}_{j}")
                    nc.vector.tensor_mul(t1[:], q_raw[:], cs_sb[:, ts])
                    qsw = rp.tile([128, TC], BF16, tag="qsw", name=f"qsw{tci}_{j}")
                    nc.gpsimd.dma_start(qsw[0:64, :], q_raw[64:128, :])
                    nc.gpsimd.dma_start(qsw[64:128, :], q_raw[0:64, :])
                    t2 = rp.tile([128, TC], BF16, tag="t2", name=f"t2_{tci}_{j}")
                    nc.vector.tensor_mul(
                        t2[:], qsw[:], cs_sb[:, T + tci * TC : T + (tci + 1) * TC]
                    )
                    nc.vector.tensor_add(
                        qt_sb[:, tci * 2048 + j * TC : tci * 2048 + (j + 1) * TC],
                        t1[:],
                        t2[:],
                    )
                for ci in range(NCC):
                    nc.tensor.matmul(
                        kt_ps[:], wk_sb[:, ci * HD : (ci + 1) * HD], xs(ci),
                        start=(ci == 0), stop=(ci == NCC - 1),
                    )
                for ci in range(NCC):
                    nc.tensor.matmul(
                        vt_ps[:], wv_sb[:, ci * HD : (ci + 1) * HD], xs(ci),
                        start=(ci == 0), stop=(ci == NCC - 1),
                    )
                # rope on K
                k_raw = rp.tile([128, TC], BF16, tag="qraw", name=f"kraw{tci}")
                nc.scalar.copy(k_raw[:], kt_ps[:])
                t1k = rp.tile([128, TC], BF16, tag="t1", name=f"t1k{tci}")
                nc.vector.tensor_mul(t1k[:], k_raw[:], cs_sb[:, ts])
                ksw = rp.tile([128, TC], BF16, tag="qsw", name=f"ksw{tci}")
                nc.gpsimd.dma_start(ksw[0:64, :], k_raw[64:128, :])
                nc.gpsimd.dma_start(ksw[64:128, :], k_raw[0:64, :])
                t2k = rp.tile([128, TC], BF16, tag="t2", name=f"t2k{tci}")
                nc.vector.tensor_mul(
                    t2k[:], ksw[:], cs_sb[:, T + tci * TC : T + (tci + 1) * TC]
                )
                nc.vector.tensor_add(kt_sb[:, ts], t1k[:], t2k[:])

                # V: [d, t] psum -> sbuf bf16, then PE-transpose to [t, d]
                vt_f = rp.tile([128, TC], BF16, tag="vtf", name=f"vtf{tci}")
                nc.scalar.copy(vt_f[:], vt_ps[:])
                for s in range(TC // 128):
                    kt_i = tci * (TC // 128) + s
                    tp_ps = tp_ps_pool.tile([128, 128], BF16, tag="tp", name=f"tp{kt_i}")
                    nc.tensor.transpose(
                        tp_ps[:], vt_f[:, s * 128 : (s + 1) * 128], ident_sb
                    )
                    nc.scalar.copy(v_sb[:, kt_i * HD : (kt_i + 1) * HD], tp_ps[:])

        # -------- phase B: attention + out-projection per q-block --------
        with ExitStack() as pb:
            st_pool = pb.enter_context(tc.tile_pool(name="stps", bufs=2, space="PSUM"))
            ot_ps_pool = pb.enter_context(tc.tile_pool(name="otps", bufs=2, space="PSUM"))
            s_ps_pool = pb.enter_context(tc.tile_pool(name="sps", bufs=2, space="PSUM"))
            pt_pool = pb.enter_context(tc.tile_pool(name="pt", bufs=6))
            nrm_pool = pb.enter_context(tc.tile_pool(name="nrm", bufs=3))
            ot_sb_pool = pb.enter_context(tc.tile_pool(name="otsb", bufs=2))
            yb_pool = pb.enter_context(tc.tile_pool(name="yb", bufs=3))

            for qb in range(NTC):
                nkt = (qb + 1) * (TC // 128)
                ot_qb = ot_sb_pool.tile([128, G * TC], BF16, tag="ot", name=f"ot{qb}")

                def emit_pend(p):
                    # den/ot matmuls for a pending (hg, kt); when it closes an
                    # hg group, also emit that group's normalization.
                    hg, pkt, ppt, ot_ps, sb_ps = p
                    pst, psp = (pkt == 0), (pkt == nkt - 1)
                    for hh in range(2):
                        nc.tensor.matmul(
                            sb_ps[hh][:], ones_sb, ppt[:, hh : hh + 1, :],
                            start=pst, stop=psp,
                        )
                        nc.tensor.matmul(
                            ot_ps[hh][:],
                            v_sb[:, pkt * HD : (pkt + 1) * HD],
                            ppt[:, hh : hh + 1, :],
                            start=pst, stop=psp,
                        )
                    if psp:
                        for hh in range(2):
                            h = 2 * hg + hh
                            r_f = nrm_pool.tile([128, TC], F32, tag="rf", name=f"rf{qb}_{h}")
                            nc.vector.reciprocal_approx_fast(r_f[:], sb_ps[hh][:])
                            nc.vector.tensor_mul(
                                ot_qb[:, h * TC : (h + 1) * TC], ot_ps[hh][:], r_f[:]
                            )

                # pipelined emission carried ACROSS the head-pair groups:
                # scores(i) always precede den/ot(i-1) in the PE stream, so
                # both the group-drain and next-group-priming bubbles are
                # covered with ready matmul work.
                pend = None  # (hg, kt, pt, ot_ps, sb_ps)
                for hg in range(G // 2):
                    ot_ps = [
                        ot_ps_pool.tile([128, TC], F32, tag="otps", name=f"otps{qb}_{hg}_{hh}")
                        for hh in range(2)
                    ]
                    sb_ps = [
                        s_ps_pool.tile([128, TC], F32, tag="sps", name=f"sps{qb}_{hg}_{hh}")
                        for hh in range(2)
                    ]
                    for kt in range(nkt):
                        dj = kt - 4 * qb
                        f0 = max(dj, 0) * 128
                        s_pair = st_pool.tile(
                            [128, 2, TC], F32, tag="st", name=f"st{qb}_{hg}_{kt}"
                        )
                        for hh in range(2):
                            h = 2 * hg + hh
                            nc.tensor.matmul(
                                s_pair[:, hh : hh + 1, f0:TC],
                                kt_sb[:, kt * 128 : (kt + 1) * 128],
                                qt_sb[:, qb * 2048 + h * TC + f0 : qb * 2048 + (h + 1) * TC],
                                start=True,
                                stop=True,
                            )
                        if dj >= 0:
                            for hh in range(2):
                                nc.vector.tensor_add(
                                    s_pair[:, hh : hh + 1, f0 : f0 + 128],
                                    s_pair[:, hh : hh + 1, f0 : f0 + 128],
                                    mask_sb[:],
                                )
                        pt = pt_pool.tile([128, 2, TC], BF16, tag="pt", name=f"pt{qb}_{hg}_{kt}")
                        if f0 > 0:
                            nc.vector.memset(pt[:, :, 0:f0], 0.0)
                        nc.scalar.activation(
                            pt[:, :, f0:TC],
                            s_pair[:, :, f0:TC],
                            mybir.ActivationFunctionType.Exp,
                        )
                        if pend is not None:
                            emit_pend(pend)
                        pend = (hg, kt, pt, ot_ps, sb_ps)
                if pend is not None:
                    emit_pend(pend)

                # out-projection for this q-block; y rows assembled to one
                # [128, C] block per t-subtile -> single DMA each
                for tl in range(TC // 128):
                    tsub = qb * (TC // 128) + tl
                    y_blk = yb_pool.tile([128, C], BF16, tag="yb", name=f"yb{tsub}")
                    for ccp in range(2):
                        ypr = st_pool.tile([128, 2, TC], F32, tag="st", name=f"yps{tsub}_{ccp}")
                        for cch in range(2):
                            cc = 2 * ccp + cch
                            for h in range(G):
                                nc.tensor.matmul(
                                    ypr[:, cch : cch + 1, :],
                                    ot_qb[:, h * TC + tl * 128 : h * TC + (tl + 1) * 128],
                                    wo_sb[:, h * C + cc * TC : h * C + (cc + 1) * TC],
                                    start=(h == 0),
                                    stop=(h == G - 1),
                                )
                            nc.vector.tensor_copy(
                                y_blk[:, cc * TC : (cc + 1) * TC], ypr[:, cch : cch + 1, :]
                            )
                        nc.sync.dma_start(
                            y[tsub * 128 : (tsub + 1) * 128, ccp * 1024 : (ccp + 1) * 1024],
                            y_blk[:, ccp * 1024 : (ccp + 1) * 1024],
                        )

    nc.compile()
    return nc


def _rope_tables():
    theta = 1.0 / (10000.0 ** (np.arange(0, HD, 2, dtype=np.float32) / HD))
    freqs = np.arange(T, dtype=np.float32)[:, None] * theta[None, :]  # [T, 64]
    cos = np.concatenate([np.cos(freqs), np.cos(freqs)], axis=-1)  # [T, 128]
    sin = np.concatenate([np.sin(freqs), np.sin(freqs)], axis=-1)
    cosT = np.ascontiguousarray(cos.T).astype(np.float32)  # [128, T]
    sinT = np.ascontiguousarray(sin.T).astype(np.float32)
    sign = np.where(np.arange(HD) < HD // 2, np.float32(-1.0), np.float32(1.0))[:, None]
    return cosT, (sinT * sign).astype(np.float32)


def _mask():
    p = np.arange(128)[:, None]
    f = np.arange(128)[None, :]
    return np.where(p <= f, 0.0, MASK_NEG).astype(np.float32)


def make_in_maps(x, Wq, Wk, Wv, Wo):
    x = np.asarray(x, dtype=np.float32)
    Wq = np.asarray(Wq, dtype=np.float32)
    Wk = np.asarray(Wk, dtype=np.float32)
    Wv = np.asarray(Wv, dtype=np.float32)
    Wo = np.asarray(Wo, dtype=np.float32)

    cosT, sinTs = _rope_tables()
    cs_host = np.ascontiguousarray(
        np.concatenate([cosT, sinTs], axis=1)
    ).astype(BFNP)
    maskf = _mask()
    consts = np.ascontiguousarray(
        np.concatenate([np.eye(128, dtype=np.float32), np.ones((128, 128), np.float32)], axis=1)
    ).astype(BFNP)
    qscale = np.float32(1.0 / np.sqrt(HD))

    in_maps = []
    for c in range(N_CORES):
        b, g = divmod(c, N_KV_HEADS)
        xT = np.ascontiguousarray(x[b].T)  # [C, T]
        xp = np.ascontiguousarray(
            xT.reshape(NCC, 128, NTC, TC).transpose(1, 2, 0, 3).reshape(128, NTC * NCC * TC)
        ).astype(BFNP)
        wq_h = np.ascontiguousarray(
            (Wq[:, g * GW : (g + 1) * GW] * qscale)
            .reshape(NCC, 128, GW).transpose(1, 0, 2).reshape(128, NCC * GW)
        ).astype(BFNP)
        wk_h = np.ascontiguousarray(
            Wk[:, g * HD : (g + 1) * HD].reshape(NCC, 128, HD).transpose(1, 0, 2).reshape(128, NCC * HD)
        ).astype(BFNP)
        wv_h = np.ascontiguousarray(
            Wv[:, g * HD : (g + 1) * HD].reshape(NCC, 128, HD).transpose(1, 0, 2).reshape(128, NCC * HD)
        ).astype(BFNP)
        wo_h = np.ascontiguousarray(
            Wo[g * GW : (g + 1) * GW, :].reshape(G, 128, C).transpose(1, 0, 2).reshape(128, G * C)
        ).astype(BFNP)
        in_maps.append(
            {
                "xp": xp,
                "wq": wq_h,
                "wk": wk_h,
                "wv": wv_h,
                "wo": wo_h,
                "cs": cs_host,
                "maskf": maskf,
                "consts": consts,
            }
        )
    return in_maps


def kernel(x, Wq, Wk, Wv, Wo):
    if "nc" not in _prog_cache:
        _prog_cache["nc"] = _build_program()
    nc = _prog_cache["nc"]

    in_maps = make_in_maps(x, Wq, Wk, Wv, Wo)
    res = run_bass_kernel_spmd(nc, in_maps, list(range(N_CORES)))
    _prog_cache["last_results"] = res

    out = np.zeros((B, T, C), dtype=np.float32)
    for c in range(N_CORES):
        b = c // N_KV_HEADS
        out[b] += res.results[c]["y"].astype(np.float32)
    return out


# revision 12
# speedup vs baseline: 1.1555x; 1.1555x over previous
"""Causal self-attention (GQA + RoPE) Trainium2 kernel, v2 (bf16).

Full-input contract: kernel(**inputs) takes the unsharded tensors and returns
the full [B, T, C] output. Shards over 8 NeuronCores as (batch b) x (kv-head
group g); each core computes the attention output of its 4 query heads for its
batch plus the partial out-projection against its 512 rows of Wo; the host
upcasts the bf16 partials and sums the 4 group partials per batch.

All matmul operands are bf16 (fp32 PSUM accumulation). The host pre-arranges
every input into its final SBUF layout so each tensor loads with one or two
large contiguous DMAs. Phase A runs j-major accumulation passes (16 chunk
matmuls per output tile) so PSUM tiles free up a full pass before they are
reallocated. Phase B emits scores(kt+1) before ones/ot(kt) so the PE never
head-of-line blocks on the exp, and processes head pairs through shared
[128,2,512] PSUM tiles so one ACT instruction exps both heads.
"""

import sys

for _p in ("/opt/trn_rl_repo", "/root/.axon_site/_ro/trn_rl_repo"):
    if _p not in sys.path:
        sys.path.append(_p)

import numpy as np
import ml_dtypes
from contextlib import ExitStack

import concourse.bass as bass
import concourse.bacc as bacc
import concourse.tile as tile
import concourse.mybir as mybir
from concourse.bass_utils import run_bass_kernel_spmd

F32 = mybir.dt.float32
BF16 = mybir.dt.bfloat16
BFNP = ml_dtypes.bfloat16

B, T, C = 2, 2048, 2048
N_HEADS, N_KV_HEADS, HD = 16, 4, 128
G = N_HEADS // N_KV_HEADS  # 4 heads per kv group
GW = G * HD  # 512
N_CORES = 8
TC = 512  # q-block / t-chunk width
NTC = T // TC  # 4
NCC = C // 128  # 16 contraction chunks
MASK_NEG = -1.0e30

_prog_cache = {}


def _build_program():
    nc = bacc.Bacc(
        "TRN2",
        target_bir_lowering=False,
        debug=False,
        enable_asserts=False,
        num_devices=N_CORES,
    )

    xp = nc.dram_tensor("xp", [128, NTC * NCC * TC], BF16, kind="ExternalInput").ap()
    wq = nc.dram_tensor("wq", [128, NCC * GW], BF16, kind="ExternalInput").ap()
    wk = nc.dram_tensor("wk", [128, NCC * HD], BF16, kind="ExternalInput").ap()
    wv = nc.dram_tensor("wv", [128, NCC * HD], BF16, kind="ExternalInput").ap()
    wo = nc.dram_tensor("wo", [128, G * C], BF16, kind="ExternalInput").ap()
    cs = nc.dram_tensor("cs", [128, 2 * T], BF16, kind="ExternalInput").ap()
    maskf = nc.dram_tensor("maskf", [128, 128], F32, kind="ExternalInput").ap()
    consts = nc.dram_tensor("consts", [128, 256], BF16, kind="ExternalInput").ap()
    y = nc.dram_tensor("y", [T, C], BF16, kind="ExternalOutput").ap()

    with tile.TileContext(nc) as tc, ExitStack() as ctx:
        cpool = ctx.enter_context(tc.tile_pool(name="const", bufs=1))
        big = ctx.enter_context(tc.tile_pool(name="big", bufs=1))

        wq_sb = cpool.tile([128, NCC * GW], BF16)
        wk_sb = cpool.tile([128, NCC * HD], BF16)
        wv_sb = cpool.tile([128, NCC * HD], BF16)
        wo_sb = cpool.tile([128, G * C], BF16)
        cs_sb = cpool.tile([128, 2 * T], BF16)
        mask_sb = cpool.tile([128, 128], F32)
        consts_sb = cpool.tile([128, 256], BF16)
        warm_sb = cpool.tile([128, 512], BF16)

        x_sb = big.tile([128, NTC * NCC * TC], BF16)  # per t-chunk, chunk-major
        qt_sb = big.tile([128, G * T], BF16)  # qb-major: qb*2048 + j*512 + t
        kt_sb = big.tile([128, T], BF16)
        v_sb = big.tile([128, T], BF16)  # kt-major blocks of [t, d]

        ident_sb = consts_sb[:, 0:128]
        ones_sb = consts_sb[:, 128:256]

        # ---------------- prologue: warm-up + input DMAs ----------------
        # Weights go on the gpsimd (SWDGE) queue: the scalar/ACT queue must
        # stay free for rope copies (an ACT-queued DMA serializes with its
        # transfer and delays every PSUM-ring free downstream). x streams on
        # sync with fine-grained leading pieces so the first j-pass never
        # outruns the transfer.
        nc.vector.memset(warm_sb[:], 0.0)
        nc.scalar.dma_start(wq_sb[:, 0:GW], wq[:, 0:GW])
        nc.scalar.dma_start(wk_sb[:], wk[:])
        nc.scalar.dma_start(wv_sb[:], wv[:])
        nc.scalar.dma_start(wq_sb[:, GW:], wq[:, GW:])
        nc.scalar.dma_start(cs_sb[:], cs[:])
        nc.scalar.dma_start(mask_sb[:], maskf[:])
        nc.scalar.dma_start(consts_sb[:], consts[:])
        nc.scalar.dma_start(wo_sb[:], wo[:])
        xsplits = [0, 512, 2048, 4096, 6144, 8192, 12288, 16384, 24576, 32768]
        for a, b_ in zip(xsplits[:-1], xsplits[1:]):
            nc.sync.dma_start(x_sb[:, a:b_], xp[:, a:b_])

        # ---------------- phase A: projections + rope ----------------
        with ExitStack() as pa:
            qt_ps_pool = pa.enter_context(tc.tile_pool(name="qtps", bufs=4, space="PSUM"))
            kv_ps_pool = pa.enter_context(tc.tile_pool(name="kvps", bufs=2, space="PSUM"))
            tp_ps_pool = pa.enter_context(tc.tile_pool(name="tpps", bufs=1, space="PSUM"))
            rp = pa.enter_context(tc.tile_pool(name="rp", bufs=3))

            # warm-up matmuls: no DMA dependency, flip the HAM clock gate
            # while the first input DMAs land. Reuse the qt psum ring.
            for w in range(10):
                wt = qt_ps_pool.tile([128, TC], F32, tag="qtps", name=f"wm{w}")
                nc.tensor.matmul(wt[:], warm_sb[:, 0:128], warm_sb[:], start=True, stop=True)

            for tci in range(NTC):
                ts = slice(tci * TC, (tci + 1) * TC)
                qt_ps = [
                    qt_ps_pool.tile([128, TC], F32, tag="qtps", name=f"qtps{tci}_{j}")
                    for j in range(G)
                ]
                kt_ps = kv_ps_pool.tile([128, TC], F32, tag="kvps", name=f"ktps{tci}")
                vt_ps = kv_ps_pool.tile([128, TC], F32, tag="kvps", name=f"vtps{tci}")

                def xs(ci):
                    base = tci * NCC * TC
                    return x_sb[:, base + ci * TC : base + (ci + 1) * TC]

                # j-major passes: each PSUM tile's 16-chunk accumulation is a
                # contiguous run, so the previous t-chunk's rope copies have a
                # full pass of slack before their tiles are reused.
                for j in range(G):
                    for ci in range(NCC):
                        nc.tensor.matmul(
                            qt_ps[j][:],
                            wq_sb[:, ci * GW + j * HD : ci * GW + (j + 1) * HD],
                            xs(ci),
                            start=(ci == 0),
                            stop=(ci == NCC - 1),
                        )
                for ci in range(NCC):
                    nc.tensor.matmul(
                        kt_ps[:], wk_sb[:, ci * HD : (ci + 1) * HD], xs(ci),
                        start=(ci == 0), stop=(ci == NCC - 1),
                    )
                for ci in range(NCC):
                    nc.tensor.matmul(
                        vt_ps[:], wv_sb[:, ci * HD : (ci + 1) * HD], xs(ci),
                        start=(ci == 0), stop=(ci == NCC - 1),
                    )

                # rope on Q heads: out = q*cos + swap(q)*sin_signed (bf16 DVE)
                for j in range(G):
                    q_raw = rp.tile([128, TC], BF16, tag="qraw", name=f"qraw{tci}_{j}")
                    nc.scalar.copy(q_raw[:], qt_ps[j][:])
                    t1 = rp.tile([128, TC], BF16, tag="t1", name=f"t1_{tci}_{j}")
                    nc.vector.tensor_mul(t1[:], q_raw[:], cs_sb[:, ts])
                    qsw = rp.tile([128, TC], BF16, tag="qsw", name=f"qsw{tci}_{j}")
                    nc.gpsimd.dma_start(qsw[0:64, :], q_raw[64:128, :])
                    nc.gpsimd.dma_start(qsw[64:128, :], q_raw[0:64, :])
                    t2 = rp.tile([128, TC], BF16, tag="t2", name=f"t2_{tci}_{j}")
                    nc.vector.tensor_mul(
                        t2[:], qsw[:], cs_sb[:, T + tci * TC : T + (tci + 1) * TC]
                    )
                    nc.vector.tensor_add(
                        qt_sb[:, tci * 2048 + j * TC : tci * 2048 + (j + 1) * TC],
                        t1[:],
                        t2[:],
                    )
                # rope on K
                k_raw = rp.tile([128, TC], BF16, tag="qraw", name=f"kraw{tci}")
                nc.scalar.copy(k_raw[:], kt_ps[:])
                t1k = rp.tile([128, TC], BF16, tag="t1", name=f"t1k{tci}")
                nc.vector.tensor_mul(t1k[:], k_raw[:], cs_sb[:, ts])
                ksw = rp.tile([128, TC], BF16, tag="qsw", name=f"ksw{tci}")
                nc.gpsimd.dma_start(ksw[0:64, :], k_raw[64:128, :])
                nc.gpsimd.dma_start(ksw[64:128, :], k_raw[0:64, :])
                t2k = rp.tile([128, TC], BF16, tag="t2", name=f"t2k{tci}")
                nc.vector.tensor_mul(
                    t2k[:], ksw[:], cs_sb[:, T + tci * TC : T + (tci + 1) * TC]
                )
                nc.vector.tensor_add(kt_sb[:, ts], t1k[:], t2k[:])

                # V: [d, t] psum -> sbuf bf16, then PE-transpose to [t, d]
                vt_f = rp.tile([128, TC], BF16, tag="vtf", name=f"vtf{tci}")
                nc.scalar.copy(vt_f[:], vt_ps[:])
                for s in range(TC // 128):
                    kt_i = tci * (TC // 128) + s
                    tp_ps = tp_ps_pool.tile([128, 128], BF16, tag="tp", name=f"tp{kt_i}")
                    nc.tensor.transpose(
                        tp_ps[:], vt_f[:, s * 128 : (s + 1) * 128], ident_sb
                    )
                    nc.scalar.copy(v_sb[:, kt_i * HD : (kt_i + 1) * HD], tp_ps[:])

        # -------- phase B: attention + out-projection per q-block --------
        with ExitStack() as pb:
            st_pool = pb.enter_context(tc.tile_pool(name="stps", bufs=2, space="PSUM"))
            ot_ps_pool = pb.enter_context(tc.tile_pool(name="otps", bufs=2, space="PSUM"))
            s_ps_pool = pb.enter_context(tc.tile_pool(name="sps", bufs=2, space="PSUM"))
            pt_pool = pb.enter_context(tc.tile_pool(name="pt", bufs=6))
            nrm_pool = pb.enter_context(tc.tile_pool(name="nrm", bufs=3))
            ot_sb_pool = pb.enter_context(tc.tile_pool(name="otsb", bufs=2))
            yb_pool = pb.enter_context(tc.tile_pool(name="yb", bufs=3))

            for qb in range(NTC):
                nkt = (qb + 1) * (TC // 128)
                ot_qb = ot_sb_pool.tile([128, G * TC], BF16, tag="ot", name=f"ot{qb}")
                for hg in range(G // 2):
                    ot_ps = [
                        ot_ps_pool.tile([128, TC], F32, tag="otps", name=f"otps{qb}_{hg}_{hh}")
                        for hh in range(2)
                    ]
                    sb_ps = [
                        s_ps_pool.tile([128, TC], F32, tag="sps", name=f"sps{qb}_{hg}_{hh}")
                        for hh in range(2)
                    ]

                    # software-pipelined emission: scores(kt) then ones/ot(kt-1)
                    # so the PE never queues behind the exp of the current kt.
                    pend = None  # (kt, pt tile)
                    for kt in range(nkt):
                        dj = kt - 4 * qb
                        f0 = max(dj, 0) * 128
                        s_pair = st_pool.tile(
                            [128, 2, TC], F32, tag="st", name=f"st{qb}_{hg}_{kt}"
                        )
                        for hh in range(2):
                            h = 2 * hg + hh
                            nc.tensor.matmul(
                                s_pair[:, hh : hh + 1, f0:TC],
                                kt_sb[:, kt * 128 : (kt + 1) * 128],
                                qt_sb[:, qb * 2048 + h * TC + f0 : qb * 2048 + (h + 1) * TC],
                                start=True,
                                stop=True,
                            )
                        if dj >= 0:
                            for hh in range(2):
                                nc.vector.tensor_add(
                                    s_pair[:, hh : hh + 1, f0 : f0 + 128],
                                    s_pair[:, hh : hh + 1, f0 : f0 + 128],
                                    mask_sb[:],
                                )
                        pt = pt_pool.tile([128, 2, TC], BF16, tag="pt", name=f"pt{qb}_{hg}_{kt}")
                        if f0 > 0:
                            nc.vector.memset(pt[:, :, 0:f0], 0.0)
                        nc.scalar.activation(
                            pt[:, :, f0:TC],
                            s_pair[:, :, f0:TC],
                            mybir.ActivationFunctionType.Exp,
                        )
                        if pend is not None:
                            pkt, ppt = pend
                            pst, psp = (pkt == 0), (pkt == nkt - 1)
                            for hh in range(2):
                                nc.tensor.matmul(
                                    sb_ps[hh][:], ones_sb, ppt[:, hh : hh + 1, :],
                                    start=pst, stop=psp,
                                )
                                nc.tensor.matmul(
                                    ot_ps[hh][:],
                                    v_sb[:, pkt * HD : (pkt + 1) * HD],
                                    ppt[:, hh : hh + 1, :],
                                    start=pst, stop=psp,
                                )
                        pend = (kt, pt)
                    pkt, ppt = pend
                    pst, psp = (pkt == 0), (pkt == nkt - 1)
                    for hh in range(2):
                        nc.tensor.matmul(
                            sb_ps[hh][:], ones_sb, ppt[:, hh : hh + 1, :],
                            start=pst, stop=psp,
                        )
                        nc.tensor.matmul(
                            ot_ps[hh][:],
                            v_sb[:, pkt * HD : (pkt + 1) * HD],
                            ppt[:, hh : hh + 1, :],
                            start=pst, stop=psp,
                        )

                    for hh in range(2):
                        h = 2 * hg + hh
                        r_f = nrm_pool.tile([128, TC], F32, tag="rf", name=f"rf{qb}_{h}")
                        nc.vector.reciprocal_approx_fast(r_f[:], sb_ps[hh][:])
                        nc.vector.tensor_mul(
                            ot_qb[:, h * TC : (h + 1) * TC], ot_ps[hh][:], r_f[:]
                        )

                # out-projection for this q-block; y rows assembled to one
                # [128, C] block per t-subtile -> single DMA each
                for tl in range(TC // 128):
                    tsub = qb * (TC // 128) + tl
                    last_blk = (qb == NTC - 1) and (tl == TC // 128 - 1)
                    y_blk = yb_pool.tile([128, C], BF16, tag="yb", name=f"yb{tsub}")
                    for ccp in range(2):
                        ypr = st_pool.tile([128, 2, TC], F32, tag="st", name=f"yps{tsub}_{ccp}")
                        for cch in range(2):
                            cc = 2 * ccp + cch
                            for h in range(G):
                                nc.tensor.matmul(
                                    ypr[:, cch : cch + 1, :],
                                    ot_qb[:, h * TC + tl * 128 : h * TC + (tl + 1) * 128],
                                    wo_sb[:, h * C + cc * TC : h * C + (cc + 1) * TC],
                                    start=(h == 0),
                                    stop=(h == G - 1),
                                )
                            # final block: split the drain chain across engines
                            # so the two closing copies/DMAs overlap
                            ceng = nc.scalar if (last_blk and cch == 1) else nc.vector
                            if ceng is nc.vector:
                                ceng.tensor_copy(
                                    y_blk[:, cc * TC : (cc + 1) * TC], ypr[:, cch : cch + 1, :]
                                )
                            else:
                                ceng.copy(
                                    y_blk[:, cc * TC : (cc + 1) * TC], ypr[:, cch : cch + 1, :]
                                )
                        deng = nc.scalar if (last_blk and ccp == 1) else nc.sync
                        deng.dma_start(
                            y[tsub * 128 : (tsub + 1) * 128, ccp * 1024 : (ccp + 1) * 1024],
                            y_blk[:, ccp * 1024 : (ccp + 1) * 1024],
                        )

    nc.compile()
    return nc


def _rope_tables():
    theta = 1.0 / (10000.0 ** (np.arange(0, HD, 2, dtype=np.float32) / HD))
    freqs = np.arange(T, dtype=np.float32)[:, None] * theta[None, :]  # [T, 64]
    cos = np.concatenate([np.cos(freqs), np.cos(freqs)], axis=-1)  # [T, 128]
    sin = np.concatenate([np.sin(freqs), np.sin(freqs)], axis=-1)
    cosT = np.ascontiguousarray(cos.T).astype(np.float32)  # [128, T]
    sinT = np.ascontiguousarray(sin.T).astype(np.float32)
    sign = np.where(np.arange(HD) < HD // 2, np.float32(-1.0), np.float32(1.0))[:, None]
    return cosT, (sinT * sign).astype(np.float32)


def _mask():
    p = np.arange(128)[:, None]
    f = np.arange(128)[None, :]
    return np.where(p <= f, 0.0, MASK_NEG).astype(np.float32)


def make_in_maps(x, Wq, Wk, Wv, Wo):
    x = np.asarray(x, dtype=np.float32)
    Wq = np.asarray(Wq, dtype=np.float32)
    Wk = np.asarray(Wk, dtype=np.float32)
    Wv = np.asarray(Wv, dtype=np.float32)
    Wo = np.asarray(Wo, dtype=np.float32)

    cosT, sinTs = _rope_tables()
    cs_host = np.ascontiguousarray(
        np.concatenate([cosT, sinTs], axis=1)
    ).astype(BFNP)
    maskf = _mask()
    consts = np.ascontiguousarray(
        np.concatenate([np.eye(128, dtype=np.float32), np.ones((128, 128), np.float32)], axis=1)
    ).astype(BFNP)
    qscale = np.float32(1.0 / np.sqrt(HD))

    in_maps = []
    for c in range(N_CORES):
        b, g = divmod(c, N_KV_HEADS)
        xT = np.ascontiguousarray(x[b].T)  # [C, T]
        xp = np.ascontiguousarray(
            xT.reshape(NCC, 128, NTC, TC).transpose(1, 2, 0, 3).reshape(128, NTC * NCC * TC)
        ).astype(BFNP)
        wq_h = np.ascontiguousarray(
            (Wq[:, g * GW : (g + 1) * GW] * qscale)
            .reshape(NCC, 128, GW).transpose(1, 0, 2).reshape(128, NCC * GW)
        ).astype(BFNP)
        wk_h = np.ascontiguousarray(
            Wk[:, g * HD : (g + 1) * HD].reshape(NCC, 128, HD).transpose(1, 0, 2).reshape(128, NCC * HD)
        ).astype(BFNP)
        wv_h = np.ascontiguousarray(
            Wv[:, g * HD : (g + 1) * HD].reshape(NCC, 128, HD).transpose(1, 0, 2).reshape(128, NCC * HD)
        ).astype(BFNP)
        wo_h = np.ascontiguousarray(
            Wo[g * GW : (g + 1) * GW, :].reshape(G, 128, C).transpose(1, 0, 2).reshape(128, G * C)
        ).astype(BFNP)
        in_maps.append(
            {
                "xp": xp,
                "wq": wq_h,
                "wk": wk_h,
                "wv": wv_h,
                "wo": wo_h,
                "cs": cs_host,
                "maskf": maskf,
                "consts": consts,
            }
        )
    return in_maps


def kernel(x, Wq, Wk, Wv, Wo):
    if "nc" not in _prog_cache:
        _prog_cache["nc"] = _build_program()
    nc = _prog_cache["nc"]

    in_maps = make_in_maps(x, Wq, Wk, Wv, Wo)
    res = run_bass_kernel_spmd(nc, in_maps, list(range(N_CORES)))
    _prog_cache["last_results"] = res

    out = np.zeros((B, T, C), dtype=np.float32)
    for c in range(N_CORES):
        b = c // N_KV_HEADS
        out[b] += res.results[c]["y"].astype(np.float32)
    return out
